# revision 1
# baseline (speedup 1.0000x reference)
"""Trainium2 Bass kernel for nn_ContrastiveLearningLoss.

Strategy (data-parallel over the flattened region axis N = max_num*B = 40):
  - Each of 8 cores gets 5 regions: slabs of features_q/features_k reshaped
    to (40, 256, 11264) and the mask reshaped to (40, 11264) (flatten orders
    intentionally differ in the reference, but both are plain reshapes of
    their own tensors, so pairing region n of each is exactly the reference
    pairing).
  - On-device, each core computes masked sums s[n, c] = sum_hw f[n,c,hw]*m[n,hw]
    for q and k.  The mask row (stored uint8 to keep the 128x re-read cheap,
    ~6% extra HBM) is broadcast+cast to all 128 partitions by one SWDGE DMA,
    and the masked multiply+reduce is a single fused DVE scalar_tensor_tensor
    (out = (f*1)*mask, accum_out = row sums) per (stream, chunk).
  - This walrus build allows at most ONE sync-wait per compute instruction, so
    a tiny "guard" copy on DVE absorbs the mask-broadcast dependency before
    each group's 4 STT ops; each STT then only waits on its own feature DMA.
  - The tiny (40, 256) epilogue (means, normalize, 40x40 similarity, CE)
    runs on host in float32.
"""

import numpy as np

MAX_NUM, B, C, H, W = 10, 4, 256, 64, 176
HW = H * W          # 11264
N = MAX_NUM * B     # 40
N_CORES = 8
R = N // N_CORES    # 5 regions per core
TAU = 0.07
EPS = 1e-12

# per-region hw chunks; each chunk is one f-tile group and one mask broadcast
FGROUPS = [(0, 4096), (4096, 4096), (8192, 3072)]
N_CHUNKS = len(FGROUPS)

_CACHE = {}


def _split_multi_waits(bir_bytes):
    """Legalize the BIR for this walrus build, which encodes at most ONE
    sync-wait per instruction: any instruction carrying N>1 waits gets N-1
    preceding same-engine Drain carriers, one wait each (same semantics —
    the engine executes them in order before the instruction)."""
    import json

    m = json.loads(bir_bytes)
    k = 0
    for fn in m["functions"]:
        for bb in fn["blocks"]:
            out = []
            for inst in bb["instructions"]:
                si = inst.get("sync_info")
                waits = (si or {}).get("on_wait") or []
                if len(waits) > 1:
                    for w in waits[:-1]:
                        k += 1
                        carrier = {
                            "engine": inst["engine"],
                            "ins": [],
                            "outs": [],
                            "name": f"{inst['name']}-sw{k}",
                            "opcode": "Drain",
                            "sync_info": {"on_update": [], "on_wait": [w]},
                        }
                        if "debug" in inst:
                            carrier["debug"] = inst["debug"]
                        out.append(carrier)
                    si["on_wait"] = [waits[-1]]
                out.append(inst)
            bb["instructions"] = out
    return json.dumps(m).encode()


def _build_bass():
    import concourse.bass as bass
    import concourse.tile as tile
    from concourse import mybir

    nc = bass.Bass(trn_type="TRN2")
    f32 = mybir.dt.float32
    fq = nc.dram_tensor("fq", (R, C, HW), f32, kind="ExternalInput")
    fk = nc.dram_tensor("fk", (R, C, HW), f32, kind="ExternalInput")
    mk = nc.dram_tensor("mask", (R, HW), mybir.dt.uint8, kind="ExternalInput")
    out = nc.dram_tensor("out", (128, R * 4 * N_CHUNKS), f32, kind="ExternalOutput")

    with tile.TileContext(nc) as tc:
        with (
            tc.tile_pool(name="singles", bufs=1) as singles,
            tc.tile_pool(name="fpool", bufs=2) as fpool,
            tc.tile_pool(name="mpool", bufs=3) as mpool,
        ):
            junk = singles.tile([128, R * N_CHUNKS], f32, tag="junk")
            acc = singles.tile([128, R * 4 * N_CHUNKS], f32, tag="acc")

            srcs = [(fq, 0), (fq, 1), (fk, 0), (fk, 1)]
            for r in range(R):
                for g, (goff, gw) in enumerate(FGROUPS):
                    mask_b = mpool.tile([128, 4096], f32, tag="mask_b", name="mask_b")
                    mrow_d = mk[r:r + 1, goff:goff + gw]
                    mrow_bcast = bass.AP(
                        tensor=mrow_d.tensor,
                        offset=mrow_d.offset,
                        ap=[[0, 128], *mrow_d.ap[1:]],
                    )
                    # SWDGE: broadcast the uint8 row to 128 partitions and
                    # cast to f32 in one DMA.
                    nc.gpsimd.dma_start(out=mask_b[:, :gw], in_=mrow_bcast)
                    # guard: absorb the mask dependency into DVE's clock so
                    # each STT below carries only its own f-DMA wait.
                    gidx = r * N_CHUNKS + g
                    nc.vector.tensor_copy(out=junk[:, gidx:gidx + 1], in_=mask_b[:, :1])
                    for s, (src, half) in enumerate(srcs):
                        ft = fpool.tile([128, 4096], f32, tag=f"f{s}", name=f"ft{s}")
                        nc.sync.dma_start(
                            out=ft[:, :gw],
                            in_=src[r, half * 128:(half + 1) * 128, goff:goff + gw],
                        )
                        col = (r * 4 + s) * N_CHUNKS + g
                        # out is written in-place into the f tile: its last
                        # writer is the same DMA the STT already waits on, so
                        # no extra WAW wait is generated (1-wait limit).
                        nc.vector.scalar_tensor_tensor(
                            out=ft[:, :gw],
                            in0=ft[:, :gw],
                            scalar=1.0,
                            in1=mask_b[:, :gw],
                            op0=mybir.AluOpType.mult,
                            op1=mybir.AluOpType.mult,
                            accum_out=acc[:, col:col + 1],
                        )
            nc.sync.dma_start(out=out[:, :], in_=acc[:, :])

    orig_to_json = nc.to_json_bytes
    nc.to_json_bytes = lambda: _split_multi_waits(orig_to_json())
    return nc


def _get_bass():
    if "nc" not in _CACHE:
        _CACHE["nc"] = _build_bass()
    return _CACHE["nc"]


def _device_masked_sums(fq40, fk40, mk40, trace=False):
    """fq40/fk40: (40, 256, 11264) f32; mk40: (40, 11264) uint8.
    Returns sums_q, sums_k each (40, 256) f32 (and the run result object)."""
    from concourse.bass_utils import run_bass_kernel_spmd

    nc = _get_bass()
    in_maps = []
    for i in range(N_CORES):
        sl = slice(i * R, (i + 1) * R)
        in_maps.append({
            "fq": np.ascontiguousarray(fq40[sl]),
            "fk": np.ascontiguousarray(fk40[sl]),
            "mask": np.ascontiguousarray(mk40[sl]),
        })
    res = run_bass_kernel_spmd(nc, in_maps, core_ids=list(range(N_CORES)), trace=trace)
    sums_q = np.empty((N, C), dtype=np.float32)
    sums_k = np.empty((N, C), dtype=np.float32)
    for i, r in enumerate(res.results):
        o = r["out"].reshape(128, R, 4, N_CHUNKS).sum(axis=3, dtype=np.float32)
        for rr in range(R):
            n = i * R + rr
            sums_q[n, 0:128] = o[:, rr, 0]
            sums_q[n, 128:256] = o[:, rr, 1]
            sums_k[n, 0:128] = o[:, rr, 2]
            sums_k[n, 128:256] = o[:, rr, 3]
    return sums_q, sums_k, res


def _epilogue(sums_q, sums_k, cnt):
    mean_q = sums_q / cnt[:, None]
    mean_k = sums_k / cnt[:, None]
    pad = mean_k[:, 0] != 0

    nrm_q = np.maximum(np.linalg.norm(mean_q, axis=-1, keepdims=True), EPS).astype(np.float32)
    nrm_k = np.maximum(np.linalg.norm(mean_k, axis=-1, keepdims=True), EPS).astype(np.float32)
    nq = mean_q / nrm_q
    nk = mean_k / nrm_k

    sim = (nk @ nq.T).astype(np.float32)
    logits = sim / np.float32(TAU)
    m = logits.max(axis=-1, keepdims=True)
    lse = np.log(np.exp(logits - m).sum(axis=-1, keepdims=True)).astype(np.float32) + m
    logp = logits - lse
    ce = -logp[np.arange(N), np.arange(N)]
    padf = pad.astype(np.float32)
    loss = (ce * padf).sum() / padf.sum()
    return np.asarray(loss, dtype=np.float32)


def kernel(features_q, features_k, mask, _trace=False, _ret_res=False):
    fq40 = np.asarray(features_q, dtype=np.float32).reshape(N, C, HW)
    fk40 = np.asarray(features_k, dtype=np.float32).reshape(N, C, HW)
    mk40 = np.asarray(mask).astype(np.uint8).reshape(N, HW)

    sums_q, sums_k, res = _device_masked_sums(fq40, fk40, mk40, trace=_trace)
    cnt = np.maximum(mk40.sum(axis=1, dtype=np.int64).astype(np.float32), np.float32(1.0))
    loss = _epilogue(sums_q, sums_k, cnt)
    if _ret_res:
        return loss, res
    return loss



# revision 2
# speedup vs baseline: 1.0631x; 1.0631x over previous
"""Trainium2 Bass kernel for nn_ContrastiveLearningLoss.

Data-parallel over the flattened region axis N = max_num*B = 40 (5 regions
per core).  Each core computes masked sums s[n,c] = sum_hw f[n,c,hw]*m[n,hw]
for q and k; the tiny (40,256) epilogue (means, normalize, 40x40 similarity,
CE) runs on host in float32.

Per-core pipeline (DMA-bound; feature traffic 115.3 MB/core ~= 321 us at
360 GB/s, TimelineSim total ~329 us vs 442 us for the SWDGE-broadcast
baseline):
  - The mask row is DMA'd ONCE per region to SBUF partition 0 (bf16), and a
    K=1 PE matmul with an all-ones stationary vector replicates it across
    the 128 partitions into PSUM (exact 0/1 in f32).  This keeps the
    128x mask broadcast OFF the DMA engines (the old SWDGE broadcast cost
    ~80 us/core of DMA-engine occupancy writing 28.8 MB to SBUF).
  - Features stream as [128ch x 4096] tiles, double-buffered, issue
    alternating between the SP and Activation HWDGE queues; the DVE STT
    (out=(f*1)*mask, accum_out=row sums) reads the mask from PSUM
    (2048-wide chunks, PSUM pool bufs=2 = all 8 banks).
  - Tail: the last region DMAs chunk-major in chunk-width waves so the DVE
    drains with ~1-chunk lag; acc columns write back per-region (Pool
    queue) / per-src (Act queue) so only the last src's 6 columns remain
    after the final STT.

- mask-row DMAs issue from the Activation queue so the SP queue's first
  feature DMA reaches the DMA engines ~1us sooner (head).
- the last region's last feature group is DMA'd per 2048/1024-chunk, so the
  final STTs chase much smaller transfers (tail).
- acc columns are written back per-region from the (idle) Pool/SWDGE queue,
  overlapping all but the last region's 24 columns with feature traffic;
  acc is padded to 128 columns so descriptors stay >=512B.
"""

import numpy as np
import ml_dtypes

MAX_NUM, B, C, H, W = 10, 4, 256, 64, 176
HW = H * W          # 11264
N = MAX_NUM * B     # 40
N_CORES = 8
R = N // N_CORES    # 5 regions per core
TAU = 0.07
EPS = 1e-12

FGROUPS = [(0, 4096), (4096, 4096), (8192, 3072)]
PSUM_W = 2048
MM_W = 512
NCH = 6             # 2048*5 + 1024
ACC_COLS = 128      # R*4*NCH = 120, padded to 128

_CACHE = {}


def _split_multi_waits(bir_bytes):
    """Legalize the BIR for this walrus build, which encodes at most ONE
    sync-wait per instruction: any instruction carrying N>1 waits gets N-1
    preceding same-engine Drain carriers, one wait each."""
    import json

    m = json.loads(bir_bytes)
    k = 0
    for fn in m["functions"]:
        for bb in fn["blocks"]:
            out = []
            for inst in bb["instructions"]:
                si = inst.get("sync_info")
                waits = (si or {}).get("on_wait") or []
                if len(waits) > 1:
                    for w in waits[:-1]:
                        k += 1
                        carrier = {
                            "engine": inst["engine"],
                            "ins": [],
                            "outs": [],
                            "name": f"{inst['name']}-sw{k}",
                            "opcode": "Drain",
                            "sync_info": {"on_update": [], "on_wait": [w]},
                        }
                        if "debug" in inst:
                            carrier["debug"] = inst["debug"]
                        out.append(carrier)
                    si["on_wait"] = [waits[-1]]
                out.append(inst)
            bb["instructions"] = out
    return json.dumps(m).encode()


def _build_bass():
    import concourse.bass as bass
    import concourse.tile as tile
    from concourse import mybir

    nc = bass.Bass(trn_type="TRN2")
    f32 = mybir.dt.float32
    bf16 = mybir.dt.bfloat16
    fq = nc.dram_tensor("fq", (R, C, HW), f32, kind="ExternalInput")
    fk = nc.dram_tensor("fk", (R, C, HW), f32, kind="ExternalInput")
    mk = nc.dram_tensor("mask", (R, HW), bf16, kind="ExternalInput")
    out = nc.dram_tensor("out", (128, ACC_COLS), f32, kind="ExternalOutput")

    with tile.TileContext(nc) as tc:
        with (
            tc.tile_pool(name="singles", bufs=1) as singles,
            tc.tile_pool(name="fpool", bufs=2) as fpool,
            tc.tile_pool(name="mrows", bufs=2) as mrows,
            tc.tile_pool(name="psum", bufs=2, space="PSUM") as psum,
        ):
            ones = singles.tile([1, 128], bf16, tag="ones")
            acc = singles.tile([128, ACC_COLS], f32, tag="acc")
            nc.vector.memset(ones[:, :], 1.0)

            srcs = [(fq, 0), (fq, 1), (fk, 0), (fk, 1)]

            def fdma(s, ft, src, half, r, off, w):
                # alternate HWDGE issue queues by src parity (SP / Act)
                eng = nc.sync if s % 2 == 0 else nc.scalar
                eng.dma_start(
                    out=ft[:, off % 4096:off % 4096 + w],
                    in_=src[r, half * 128:(half + 1) * 128, off:off + w],
                )

            for r in range(R):
                last_region = r == R - 1
                mrow = mrows.tile([1, HW], bf16, tag="mrow", name="mrow")
                nc.scalar.dma_start(out=mrow[:, :], in_=mk[r:r + 1, :])
                ci = 0
                for gi, (goff, gw) in enumerate(FGROUPS):
                    fts = []
                    for s, (src, half) in enumerate(srcs):
                        ft = fpool.tile([128, 4096], f32, tag=f"f{s}", name=f"ft{s}")
                        if not last_region:
                            fdma(s, ft, src, half, r, goff, gw)
                        fts.append(ft)
                    if last_region:
                        # chunk-major chunk-width DMA waves: DVE chases each
                        # wave with ~1-chunk lag, shrinking the drain tail
                        o = 0
                        while o < gw:
                            cw = min(PSUM_W, gw - o)
                            for s, (src, half) in enumerate(srcs):
                                fdma(s, fts[s], src, half, r, goff + o, cw)
                            o += cw
                    o = 0
                    while o < gw:
                        cw = min(PSUM_W, gw - o)
                        coff = goff + o
                        pm = psum.tile([128, PSUM_W], f32, tag="pmask", name="pmask")
                        for j in range(0, cw, MM_W):
                            mw = min(MM_W, cw - j)
                            nc.tensor.matmul(
                                pm[:, j:j + mw],
                                ones[:, :128],
                                mrow[0:1, coff + j:coff + j + mw],
                            )
                        for s in range(4):
                            col = (r * 4 + s) * NCH + ci
                            ft = fts[s]
                            nc.vector.scalar_tensor_tensor(
                                out=ft[:, o:o + cw],
                                in0=ft[:, o:o + cw],
                                scalar=1.0,
                                in1=pm[:, :cw],
                                op0=mybir.AluOpType.mult,
                                op1=mybir.AluOpType.mult,
                                accum_out=acc[:, col:col + 1],
                            )
                        ci += 1
                        o += cw
                if not last_region:
                    # per-region writeback of 24 columns from the idle
                    # Pool/SWDGE queue; overlaps with next region's traffic
                    c0 = r * 4 * NCH
                    c1 = (r + 1) * 4 * NCH
                    nc.gpsimd.dma_start(out=out[:, c0:c1], in_=acc[:, c0:c1])
                else:
                    # per-src writeback: col block [96+6s, 96+6s+6) finalizes
                    # at that src's last STT; earlier srcs flush while later
                    # srcs still compute
                    for s in range(4):
                        c0 = (r * 4 + s) * NCH
                        nc.scalar.dma_start(out=out[:, c0:c0 + NCH], in_=acc[:, c0:c0 + NCH])

    orig_to_json = nc.to_json_bytes
    nc.to_json_bytes = lambda: _split_multi_waits(orig_to_json())
    return nc


def _get_bass():
    if "nc" not in _CACHE:
        _CACHE["nc"] = _build_bass()
    return _CACHE["nc"]


def _device_masked_sums(fq40, fk40, mk40, trace=False):
    from concourse.bass_utils import run_bass_kernel_spmd

    nc = _get_bass()
    in_maps = []
    for i in range(N_CORES):
        sl = slice(i * R, (i + 1) * R)
        in_maps.append({
            "fq": np.ascontiguousarray(fq40[sl]),
            "fk": np.ascontiguousarray(fk40[sl]),
            "mask": np.ascontiguousarray(mk40[sl]),
        })
    res = run_bass_kernel_spmd(nc, in_maps, core_ids=list(range(N_CORES)), trace=trace)
    sums_q = np.empty((N, C), dtype=np.float32)
    sums_k = np.empty((N, C), dtype=np.float32)
    for i, r in enumerate(res.results):
        o = r["out"][:, :R * 4 * NCH].reshape(128, R, 4, NCH).sum(axis=3, dtype=np.float32)
        for rr in range(R):
            n = i * R + rr
            sums_q[n, 0:128] = o[:, rr, 0]
            sums_q[n, 128:256] = o[:, rr, 1]
            sums_k[n, 0:128] = o[:, rr, 2]
            sums_k[n, 128:256] = o[:, rr, 3]
    return sums_q, sums_k, res


def _epilogue(sums_q, sums_k, cnt):
    mean_q = sums_q / cnt[:, None]
    mean_k = sums_k / cnt[:, None]
    pad = mean_k[:, 0] != 0

    nrm_q = np.maximum(np.linalg.norm(mean_q, axis=-1, keepdims=True), EPS).astype(np.float32)
    nrm_k = np.maximum(np.linalg.norm(mean_k, axis=-1, keepdims=True), EPS).astype(np.float32)
    nq = mean_q / nrm_q
    nk = mean_k / nrm_k

    sim = (nk @ nq.T).astype(np.float32)
    logits = sim / np.float32(TAU)
    m = logits.max(axis=-1, keepdims=True)
    lse = np.log(np.exp(logits - m).sum(axis=-1, keepdims=True)).astype(np.float32) + m
    logp = logits - lse
    ce = -logp[np.arange(N), np.arange(N)]
    padf = pad.astype(np.float32)
    loss = (ce * padf).sum() / padf.sum()
    return np.asarray(loss, dtype=np.float32)


def kernel(features_q, features_k, mask, _trace=False, _ret_res=False):
    fq40 = np.asarray(features_q, dtype=np.float32).reshape(N, C, HW)
    fk40 = np.asarray(features_k, dtype=np.float32).reshape(N, C, HW)
    mku8 = np.asarray(mask).astype(np.uint8).reshape(N, HW)
    mk40 = mku8.astype(ml_dtypes.bfloat16)

    sums_q, sums_k, res = _device_masked_sums(fq40, fk40, mk40, trace=_trace)
    cnt = np.maximum(mku8.sum(axis=1, dtype=np.int64).astype(np.float32), np.float32(1.0))
    loss = _epilogue(sums_q, sums_k, cnt)
    if _ret_res:
        return loss, res
    return loss
